# revision 1
# baseline (speedup 1.0000x reference)
"""Trainium2 Bass kernel for packed-sequence GRU decoder (nn_Decoder).

Reference semantics (T=512, B=1024, V=64, H=100):
  per step t: h = where(t < len, GRUCell(x_t, h), h)
              out_t = where(t < len, log_softmax(h @ W_out.T + b_out), 0)

Strategy:
  - Data-parallel over batch, STRIDED: core k owns lanes k, k+8, ... (128 lanes).
    lengths are sorted descending, so striding load-balances and keeps each
    core's active lanes a prefix of its lane set at every step.
  - Device layout is transposed: hidden state h kept as [H+1, 128] (ones row
    folds biases into the matmuls via augmented weights). Gates r,z share one
    PSUM tile so one Sigmoid activation covers both.
  - lengths are specialized into the program as static per-step active-lane
    counts N_t = ceil(#{b: len_b > t} / 8); frozen lanes just stop being
    computed/stored. Host zero-fills padded output positions at the end.
  - log-softmax: per-step logits matmuls accumulate into a PSUM chunk buffer
    (TC steps); exp/ln run chunk-batched so the ACT table set (sigmoid/tanh
    vs exp/ln) switches only twice per chunk.
"""

import numpy as np

T, B, V, H = 512, 1024, 64, 100
NCORES = 8
BL = B // NCORES          # 128 lanes per core
KX = V + 1                # 65: x rows + ones row
KH = H + 1                # 101: h rows + ones row
TC = 32                   # timesteps per softmax chunk

_prog_cache: dict = {}


def _build(sched, t_steps, tc_steps):
    import concourse.bass as bass
    import concourse.mybir as mybir
    from concourse import bacc, tile
    from concourse.tile_rust import add_dep_helper

    f32 = mybir.dt.float32
    AF = mybir.ActivationFunctionType
    ALU = mybir.AluOpType
    AX = mybir.AxisListType

    nc = bacc.Bacc()

    # Steer the greedy ACT table-set picker: claim Exp/Ln live only in
    # natural_log_exp_and_others and Tanh only in sigmoid_and_others, so the
    # recurrent phase (Sigmoid+Tanh) and softmax phase (Exp+Ln) each need one
    # set and a chunk costs exactly 2 table loads. Claimed membership stays a
    # subset of the real tables, so emitted programs remain correct.
    from concourse import hw_specs

    tables = hw_specs.get_activation_tables(nc.m.arch)
    _exp, _ln, _tanh = AF.Exp, AF.Ln, AF.Tanh
    for name, fns in tables.items():
        if name != "natural_log_exp_and_others":
            fns.discard(_exp)
            fns.discard(_ln)
        if name != "sigmoid_and_others":
            fns.discard(_tanh)

    xT = nc.declare_dram_parameter("xT", [t_steps, KX, BL], f32, isOutput=False)
    hT0 = nc.declare_dram_parameter("hT0", [KH, BL], f32, isOutput=False)
    WX = nc.declare_dram_parameter("WX", [KX, 3 * H], f32, isOutput=False)
    WH = nc.declare_dram_parameter("WH", [KH, 3 * H], f32, isOutput=False)
    WO = nc.declare_dram_parameter("WO", [KH, V], f32, isOutput=False)
    OUT = nc.declare_dram_parameter("out", [t_steps, BL, V], f32, isOutput=True)

    xTr = xT.rearrange("t p l -> p t l")
    OUTr = OUT.rearrange("t l v -> l t v")

    with tile.TileContext(nc) as tc:
        with (
            tc.tile_pool(name="const", bufs=1) as cpool,
            tc.tile_pool(name="xin", bufs=3) as xpool,
            tc.tile_pool(name="work", bufs=6) as wpool,
            tc.tile_pool(name="soft", bufs=3) as spool,
            tc.tile_pool(name="pgate", bufs=1, space="PSUM") as pg,
            tc.tile_pool(name="plgp", bufs=1, space="PSUM") as plgp,
        ):
            wx = cpool.tile([KX, 3 * H], f32)
            wh = cpool.tile([KH, 3 * H], f32)
            wo = cpool.tile([KH, V], f32)
            h = cpool.tile([KH, BL], f32)
            nc.sync.dma_start(wx[:], WX[:])
            nc.sync.dma_start(wh[:], WH[:])
            nc.sync.dma_start(wo[:], WO[:])
            nc.sync.dma_start(h[:], hT0[:])

            n_chunks = t_steps // tc_steps

            def load_chunk(c):
                t0 = c * tc_steps
                ncm = sched[t0]
                xb = xpool.tile([KX, tc_steps, BL], f32, tag="xb")
                nc.sync.dma_start(xb[:, :, 0:ncm], xTr[:, t0 : t0 + tc_steps, 0:ncm])
                return xb

            def emit_gate_mms(t, xbuf):
                # gate matmuls for step t into a fresh psum slot; emitted right
                # after the previous h update so x-parts run while ACT/DVE work
                # and sigmoid's deps (r_h, z_h) clear the PE queue first among
                # the h-dependent matmuls.
                n_ = sched[t]
                xt_ = xbuf[:, t % tc_steps, 0:n_]
                hs_ = h[:, 0:n_]
                # r sits right-aligned at the end of bank 1 and z left-aligned
                # at the start of bank 2 of a 2-bank tile: their accumulation
                # groups live in different banks (so BOTH x-parts run before
                # the h update lands) while sigmoid reads one contiguous [2n]
                # window across the bank boundary. i_n|h_n get their own bank
                # so their matmuls don't bank-conflict with sigmoid's read.
                # Single-buffered: next step's writers only need the slot
                # after this step's sigmoid/tanh reads, which is always sooner
                # than the h update those writers wait on anyway.
                pgb = pg.tile([H, 8 * BL], f32, tag="pgb")
                pih = pg.tile([H, 2 * BL], f32, tag="pih")
                pr_ = pgb[:, 4 * BL - n_ : 4 * BL]
                pz_ = pgb[:, 4 * BL : 4 * BL + n_]
                prz_ = pgb[:, 4 * BL - n_ : 4 * BL + n_]
                pin_ = pih[:, 0:n_]
                phn_ = pih[:, BL : BL + n_]
                nc.tensor.matmul(pr_, wx[:, 0:H], xt_, start=True, stop=False)
                nc.tensor.matmul(pz_, wx[:, H : 2 * H], xt_, start=True, stop=False)
                nc.tensor.matmul(pr_, wh[:, 0:H], hs_, start=False, stop=True)
                nc.tensor.matmul(pz_, wh[:, H : 2 * H], hs_, start=False, stop=True)
                nc.tensor.matmul(phn_, wh[:, 2 * H : 3 * H], hs_, start=True, stop=True)
                nc.tensor.matmul(pin_, wx[:, 2 * H : 3 * H], xt_, start=True, stop=True)
                return prz_, pin_, phn_

            xb_cur = load_chunk(0)
            xb_next = load_chunk(1) if n_chunks > 1 else None
            psums = emit_gate_mms(0, xb_cur)
            plg = None
            last_ln = None

            for t in range(t_steps):
                c, tl = divmod(t, tc_steps)
                n = sched[t]
                if tl == 0:
                    t0 = t
                    ncm = sched[t0]
                    plg = plgp.tile([BL, tc_steps, V], f32, tag="plg")

                prz, pin, phn = psums
                rz = wpool.tile([H, 2 * BL], f32, tag="rz")
                # split sigmoid: u only needs the r half, so pin sig_r first
                # (explicit dep stops the scheduler putting sig_z ahead of it)
                sig_r = nc.scalar.activation(rz[:, 0:n], prz[:, 0:n], AF.Sigmoid)
                sig_z = nc.scalar.activation(rz[:, n : 2 * n], prz[:, n : 2 * n], AF.Sigmoid)
                add_dep_helper(sig_z.ins, sig_r.ins, reason="sig_r unblocks u first")
                u = wpool.tile([H, BL], f32, tag="u")
                nc.vector.tensor_mul(u[:, 0:n], phn, rz[:, 0:n])
                # v = i_n + u written in place over i_n's psum slots, so tanh
                # reads PSUM (faster ACT access) and no SBUF tile is needed
                nc.vector.tensor_add(pin, pin, u[:, 0:n])
                # off-critical-chain on Pool: s = z*h_old, zb = 1-z
                s = wpool.tile([H, BL], f32, tag="s")
                nc.gpsimd.tensor_mul(s[:, 0:n], rz[:, n : 2 * n], h[0:H, 0:n])
                zb = wpool.tile([H, BL], f32, tag="zb")
                nc.gpsimd.tensor_scalar(zb[:, 0:n], rz[:, n : 2 * n], -1.0, 1.0, ALU.mult, ALU.add)
                nt = wpool.tile([H, BL], f32, tag="nt")
                nt_act = nc.scalar.activation(nt[:, 0:n], pin, AF.Tanh)
                p = wpool.tile([H, BL], f32, tag="p")
                nc.vector.tensor_mul(p[:, 0:n], zb[:, 0:n], nt[:, 0:n])
                nc.vector.tensor_add(h[0:H, 0:n], p[:, 0:n], s[:, 0:n])

                # next step's gate matmuls before this step's logits matmul
                if t + 1 < t_steps:
                    nxt_buf = xb_cur if (t + 1) // tc_steps == c else xb_next
                    psums = emit_gate_mms(t + 1, nxt_buf)

                # logits over the chunk-max lane count, not just the active n:
                # matmul cost scales with output free size (64) only, and the
                # extra lanes (stale-h garbage, host-masked) fully initialize
                # the psum chunk so no separate memset is needed
                nc.tensor.matmul(plg[0:ncm, tl, :], h[:, 0:ncm], wo[:], start=True, stop=True)

                if tl == tc_steps - 1:
                    # chunk-batched log-softmax (exp/ln table set). NOTE: do
                    # NOT pin exp/ln order against the recurrence activations —
                    # the scheduler's interleave (4 table loads/boundary) still
                    # beats strict serialization by overlapping exp under the
                    # boundary step's DVE chain (measured: pinning +19us).
                    E = spool.tile([BL, tc_steps, V], f32, tag="E")
                    nc.scalar.activation(E[0:ncm], plg[0:ncm], AF.Exp)
                    S = spool.tile([BL, tc_steps], f32, tag="S")
                    nc.vector.tensor_reduce(S[0:ncm], E[0:ncm], axis=AX.X, op=ALU.add)
                    lnS = spool.tile([BL, tc_steps], f32, tag="lnS")
                    nc.scalar.activation(lnS[0:ncm], S[0:ncm], AF.Ln)
                    ob = spool.tile([BL, tc_steps, V], f32, tag="ob")
                    nc.vector.scalar_tensor_tensor(
                        ob[0:ncm],
                        plg[0:ncm],
                        0.0,
                        lnS[0:ncm].broadcast_to([ncm, tc_steps, V]),
                        ALU.bypass,
                        ALU.subtract,
                    )
                    nc.sync.dma_start(OUTr[0:ncm, t0 : t0 + tc_steps, :], ob[0:ncm])
                    # rotate x buffers and prefetch the chunk after next
                    if c + 1 < n_chunks:
                        xb_cur = xb_next
                        xb_next = load_chunk(c + 2) if c + 2 < n_chunks else None

    nc.compile()
    return nc


def _schedule(lengths, t_steps):
    counts = (np.asarray(lengths)[None, :] > np.arange(t_steps)[:, None]).sum(axis=1)
    return tuple(max(1, int(-(-int(c) // NCORES))) for c in counts)


def _prepare(inputs):
    x = np.ascontiguousarray(np.asarray(inputs["x"], dtype=np.float32))
    h0 = np.asarray(inputs["h"], dtype=np.float32)
    lengths = np.asarray(inputs["lengths"], dtype=np.int32)
    W_ih = np.asarray(inputs["W_ih"], dtype=np.float32)
    W_hh = np.asarray(inputs["W_hh"], dtype=np.float32)
    b_ih = np.asarray(inputs["b_ih"], dtype=np.float32)
    b_hh = np.asarray(inputs["b_hh"], dtype=np.float32)
    W_out = np.asarray(inputs["W_out"], dtype=np.float32)
    b_out = np.asarray(inputs["b_out"], dtype=np.float32)

    sched = _schedule(lengths, T)
    key = (sched, T, TC)
    if key not in _prog_cache:
        _prog_cache[key] = _build(sched, T, TC)
    nc = _prog_cache[key]

    WXh = np.empty((KX, 3 * H), np.float32)
    WXh[:V] = W_ih.T
    WXh[V] = b_ih
    WHh = np.empty((KH, 3 * H), np.float32)
    WHh[:H] = W_hh.T
    WHh[H] = b_hh
    WOh = np.empty((KH, V), np.float32)
    WOh[:H] = W_out.T
    WOh[H] = b_out

    in_maps = []
    for k in range(NCORES):
        xs = x[:, k::NCORES, :]  # [T, BL, V]
        xTk = np.empty((T, KX, BL), np.float32)
        xTk[:, :V, :] = xs.transpose(0, 2, 1)
        xTk[:, V, :] = 1.0
        hTk = np.empty((KH, BL), np.float32)
        hTk[:H] = h0[0, k::NCORES, :].T
        hTk[H] = 1.0
        in_maps.append({"xT": xTk, "hT0": hTk, "WX": WXh, "WH": WHh, "WO": WOh})

    return nc, in_maps, lengths


def kernel(**inputs):
    nc, in_maps, lengths = _prepare(inputs)

    from concourse.bass_utils import run_bass_kernel_spmd

    res = run_bass_kernel_spmd(nc, in_maps, list(range(NCORES))).results

    full = np.zeros((T, B, V), dtype=np.float32)
    for k in range(NCORES):
        full[:, k::NCORES, :] = res[k]["out"]
    full[np.arange(T)[:, None] >= lengths[None, :]] = 0.0
    return full

